# revision 12
# baseline (speedup 1.0000x reference)
"""Trainium2 Bass kernel for the DNC memory-system step (nn_DNCMemorySystem).

Pure data-parallel across 8 NeuronCores: each core processes 8 of the 64
batch elements; small controller weights are replicated.  Self-contained:
shapes/sharding are hardcoded.
"""
import numpy as np

B, N, W, R, H = 64, 1024, 64, 4, 256
NCORES = 8
BL = B // NCORES          # 8 local batches per core
P = 128                   # SBUF partitions
NB = N // P               # 8 row-blocks of the NxN link matrix
IF = 471                  # interface vector size
IN_DIM = W + R * W        # 320 controller input
NEG_CLAMP = -60.0

_CACHE = {}

# interface vector offsets
_O_RK = 0
_O_WK = R * W                 # 256
_O_WS = _O_WK + W             # 320
_O_EV = _O_WS + 1             # 321
_O_WV = _O_EV + W             # 385
_O_RS = _O_WV + W             # 449
_O_RM = _O_RS + R             # 453
_O_WG = _O_RM + 3 * R         # 465
_O_AG = _O_WG + 1             # 466
_O_FG = _O_AG + 1             # 467


def _build():
    import concourse.bass as bass
    import concourse.mybir as mybir
    import concourse.tile as tile
    from concourse import bacc
    from concourse.masks import make_identity

    fp32 = mybir.dt.float32
    Alu = mybir.AluOpType
    Act = mybir.ActivationFunctionType

    nc = bacc.Bacc("TRN2", target_bir_lowering=False, debug=False,
                   num_devices=NCORES)

    din = {}
    for name, shape in [
        ("input_vector", [BL, W]), ("memory_matrix", [BL, N, W]),
        ("read_weights", [BL, R, N]), ("usage_vector", [BL, N]),
        ("link", [BL, N, N]), ("precedence", [BL, N]),
        ("read_vectors", [BL, R, W]),
        ("Wih0", [4 * H, IN_DIM]), ("bih0", [4 * H]), ("bhh0", [4 * H]),
        ("Wih1", [4 * H, H]), ("bih1", [4 * H]), ("bhh1", [4 * H]),
        ("Wih2", [4 * H, H]), ("bih2", [4 * H]), ("bhh2", [4 * H]),
        ("Wint", [IF, H]), ("bint", [IF]),
        ("Wenc", [W, W]), ("benc", [W]),
        ("Wout", [W, H + R * W]), ("bout", [W]),
    ]:
        din[name] = nc.dram_tensor(name, shape, fp32, kind="ExternalInput").ap()

    dout = {}
    for name, shape in [
        ("o_out", [BL, W]), ("o_mem", [BL, N, W]), ("o_wr", [BL, R, N]),
        ("o_ww", [BL, N]), ("o_usage", [BL, N]), ("o_Lnew", [BL, N, N]),
        ("o_pnew", [BL, N]), ("o_rv", [BL, R, W]),
    ]:
        dout[name] = nc.dram_tensor(name, shape, fp32, kind="ExternalOutput").ap()

    with tile.TileContext(nc) as tc:
        _body(nc, tc, din, dout, mybir, make_identity, fp32, Alu, Act)
    nc.compile()
    return nc


def _body(nc, tc, din, dout, mybir, make_identity, fp32, Alu, Act):
    act = nc.scalar
    dve = nc.vector
    gps = nc.gpsimd
    pe = nc.tensor
    dma = nc.sync
    AX = mybir.AxisListType

    # ---------------- persistent pools ----------------
    cpool = tc.alloc_tile_pool(name="const", bufs=1)
    pers = tc.alloc_tile_pool(name="persist", bufs=1)
    colp = tc.alloc_tile_pool(name="cols", bufs=1)
    ring = tc.alloc_tile_pool(name="ring", bufs=2)   # small transient tiles
    pp_small = tc.alloc_tile_pool(name="pp_small", bufs=2, space="PSUM")
    pp_acc = tc.alloc_tile_pool(name="pp_acc", bufs=2, space="PSUM")
    big = None  # [*, N]-sized transient pool; created after weights release

    ident = cpool.tile([P, P], fp32, name="ident")
    make_identity(nc, ident)
    ones8 = cpool.tile([1, BL], fp32, name="ones8")
    gps.memset(ones8, 1.0)
    # weights needed after the early phases live in cpool
    WencT = cpool.tile([W, W], fp32, name="WencT")
    WoutT = [cpool.tile([P, W], fp32, name=f"WoutT{c}") for c in range(4)]
    benc = cpool.tile([1, W], fp32, name="benc")
    bout = cpool.tile([1, W], fp32, name="bout")

    def tpose(out_sb, in_ap, tag):
        """PE-transpose in_ap -> psum -> ACT copy to out_sb (transposed)."""
        pp = pp_small.tile([P, P], fp32, name=f"tp_{tag}", tag="tps")
        pin, fin = in_ap.shape[0], in_ap.shape[1]
        ps = pp[:fin, :pin]
        pe.transpose(ps, in_ap, ident[:pin, :pin])
        act.copy(out_sb, ps)

    def bcast(rep_tile, src_row_ap, tag):
        """Replicate a single-partition row to rep_tile's partitions.
        partition_broadcast needs the source at partition 0 -> DMA bounce."""
        row = ring.tile([1, src_row_ap.shape[-1]], fp32, name=f"bnc_{tag}",
                        tag="bounce", bufs=3)
        dma.dma_start(out=row, in_=src_row_ap)
        gps.partition_broadcast(rep_tile, row)

    # ================= PHASE A+B: weights + controller (scoped) ==========
    wpool = tc.alloc_tile_pool(name="weights", bufs=1)
    stage = tc.alloc_tile_pool(name="stage", bufs=2)

    def load_transpose(wname, rows, cols, tag, chunk_tiles=None):
        """W [rows, cols] -> list of col-chunk tiles [csz, rows] (= W.T)."""
        nch = (cols + P - 1) // P
        if chunk_tiles is None:
            chunk_tiles = [
                wpool.tile([min(P, cols - c * P), rows], fp32, name=f"{tag}T{c}")
                for c in range(nch)]
        nrb = (rows + P - 1) // P
        for rb in range(nrb):
            rsz = min(P, rows - rb * P)
            raw = stage.tile([rsz, cols], fp32, name=f"raw_{tag}_{rb}",
                             tag="wraw")
            dma.dma_start(out=raw, in_=din[wname][rb * P: rb * P + rsz, :])
            for c in range(nch):
                csz = min(P, cols - c * P)
                t = chunk_tiles[c]
                pp = pp_small.tile([P, P], fp32, name=f"wt_{tag}_{rb}_{c}",
                                   tag="tps")
                pe.transpose(pp[:csz, :rsz], raw[:, c * P: c * P + csz],
                             ident[:rsz, :rsz])
                act.copy(t[:, rb * P: rb * P + rsz], pp[:csz, :rsz])
        return chunk_tiles

    W0T = load_transpose("Wih0", 4 * H, IN_DIM, "w0")
    W1T = load_transpose("Wih1", 4 * H, H, "w1")
    W2T = load_transpose("Wih2", 4 * H, H, "w2")
    WintT = load_transpose("Wint", IF, H, "wi")
    load_transpose("Wout", W, H + R * W, "wo", chunk_tiles=WoutT)
    load_transpose("Wenc", W, W, "we", chunk_tiles=[WencT])

    def bias_row(names, size, tag, tgt=None):
        t = tgt if tgt is not None else wpool.tile([1, size], fp32,
                                                   name=f"b_{tag}")
        dma.dma_start(out=t, in_=din[names[0]].unsqueeze(0))
        if len(names) > 1:
            t2 = stage.tile([1, size], fp32, name=f"b2_{tag}", tag="b2", bufs=1)
            dma.dma_start(out=t2, in_=din[names[1]].unsqueeze(0))
            dve.tensor_add(t, t, t2)
        return t

    bc0 = bias_row(["bih0", "bhh0"], 4 * H, "c0")
    bc1 = bias_row(["bih1", "bhh1"], 4 * H, "c1")
    bc2 = bias_row(["bih2", "bhh2"], 4 * H, "c2")
    bint = bias_row(["bint"], IF, "int")
    bias_row(["benc"], W, "enc", tgt=benc)
    bias_row(["bout"], W, "out", tgt=bout)

    x = stage.tile([BL, IN_DIM], fp32, name="x", tag="x")
    dma.dma_start(out=x[:, :W], in_=din["input_vector"])
    dma.dma_start(out=x[:, W:],
                  in_=din["read_vectors"].rearrange("b r w -> b (r w)"))

    def fc(inp_sb, in_dim, WT, brow, out_dim, tag):
        """[8, in_dim] @ W.T + b -> psum [8, out_dim]."""
        g = pp_acc.tile([BL, out_dim], fp32, name=f"g_{tag}", tag="acc")
        nkc = (in_dim + P - 1) // P
        xT = []
        for c in range(nkc):
            csz = min(P, in_dim - c * P)
            t = stage.tile([csz, BL], fp32, name=f"xT_{tag}_{c}", tag="xT", bufs=4)
            tpose(t, inp_sb[:, c * P: c * P + csz], f"{tag}{c}")
            xT.append((t, csz))
        nnc = (out_dim + 511) // 512
        for nch in range(nnc):
            nsz = min(512, out_dim - nch * 512)
            osl = g[:, nch * 512: nch * 512 + nsz]
            for c, (t, csz) in enumerate(xT):
                pe.matmul(osl, t, WT[c][:, nch * 512: nch * 512 + nsz],
                          start=(c == 0), stop=False, skip_group_check=True)
            pe.matmul(osl, ones8, brow[:, nch * 512: nch * 512 + nsz],
                      start=False, stop=True, skip_group_check=True)
        return g

    def lstm(inp_sb, in_dim, WT, brow, tag, hpool):
        g = fc(inp_sb, in_dim, WT, brow, 4 * H, tag)
        si = ring.tile([BL, H], fp32, name=f"si_{tag}", tag="rs", bufs=6)
        tg = ring.tile([BL, H], fp32, name=f"tg_{tag}", tag="rs", bufs=6)
        so = ring.tile([BL, H], fp32, name=f"so_{tag}", tag="rs", bufs=6)
        act.activation(si, g[:, 0:H], Act.Sigmoid)
        act.activation(tg, g[:, 2 * H:3 * H], Act.Tanh)
        act.activation(so, g[:, 3 * H:4 * H], Act.Sigmoid)
        c = ring.tile([BL, H], fp32, name=f"c_{tag}", tag="rs", bufs=6)
        dve.tensor_mul(c, si, tg)
        tc_ = ring.tile([BL, H], fp32, name=f"tc_{tag}", tag="rs", bufs=6)
        act.activation(tc_, c, Act.Tanh)
        h = hpool.tile([BL, H], fp32, name=f"h_{tag}", tag=f"h_{tag}")
        dve.tensor_mul(h, so, tc_)
        return h

    h1 = lstm(x, IN_DIM, W0T, bc0, "l0", stage)
    h2 = lstm(h1, H, W1T, bc1, "l1", stage)
    h3 = lstm(h2, H, W2T, bc2, "l2", pers)   # needed in phase F
    iv_ps = fc(h3, H, WintT, bint, IF, "int")
    iv = pers.tile([BL, IF], fp32, name="iv")
    act.copy(iv, iv_ps)

    stage.release()
    wpool.release()

    big = tc.alloc_tile_pool(name="big", bufs=2)

    # ---- parse interface vector (batched [8, *]) ----
    ws_t = pers.tile([BL, 1], fp32, name="ws_t")
    act.activation(ws_t, iv[:, _O_WS:_O_WS + 1], Act.Exp)
    dve.tensor_scalar_add(ws_t, ws_t, 1.0)
    act.activation(ws_t, ws_t, Act.Ln)
    ev = pers.tile([BL, W], fp32, name="ev")
    act.activation(ev, iv[:, _O_EV:_O_EV + W], Act.Sigmoid)
    rs_t = pers.tile([BL, R], fp32, name="rs_t")
    act.activation(rs_t, iv[:, _O_RS:_O_RS + R], Act.Exp)
    dve.tensor_scalar_add(rs_t, rs_t, 1.0)
    act.activation(rs_t, rs_t, Act.Ln)
    wg_t = pers.tile([BL, 1], fp32, name="wg_t")
    act.activation(wg_t, iv[:, _O_WG:_O_WG + 1], Act.Sigmoid)
    ag_t = pers.tile([BL, 1], fp32, name="ag_t")
    act.activation(ag_t, iv[:, _O_AG:_O_AG + 1], Act.Sigmoid)
    nag_t = pers.tile([BL, 1], fp32, name="nag_t")       # 1 - ag
    dve.tensor_scalar(nag_t, ag_t, -1.0, 1.0, Alu.mult, Alu.add)
    fg = pers.tile([BL, R], fp32, name="fg")
    act.activation(fg, iv[:, _O_FG:_O_FG + R], Act.Sigmoid)
    nfg = pers.tile([BL, R], fp32, name="nfg")           # -fg
    dve.tensor_scalar_mul(nfg, fg, -1.0)

    # rm softmax over inner dim of [8, 4, 3]
    rm_view = iv[:, _O_RM:_O_RM + 3 * R].rearrange("b (r k) -> b r k", k=3)
    rmx = ring.tile([BL, R], fp32, name="rmx", tag="r4", bufs=4)
    dve.tensor_reduce(rmx, rm_view, AX.X, Alu.max)
    nrmx = ring.tile([BL, R], fp32, name="nrmx", tag="r4", bufs=4)
    dve.tensor_scalar_mul(nrmx, rmx, -1.0)
    rme = ring.tile([BL, 3 * R], fp32, name="rme", tag="r12")
    for k in range(3):
        t_k = ring.tile([BL, R], fp32, name=f"rma{k}", tag="r4b", bufs=4)
        dve.tensor_add(t_k, iv[:, _O_RM:_O_RM + 3 * R][:, k::3], nrmx)
        act.activation(rme[:, k::3], t_k, Act.Exp)
    rmsum = ring.tile([BL, R], fp32, name="rmsum", tag="r4", bufs=4)
    dve.tensor_reduce(rmsum, rme.rearrange("b (r k) -> b r k", k=3),
                      AX.X, Alu.add)
    rmrec = ring.tile([BL, R], fp32, name="rmrec", tag="r4", bufs=4)
    dve.reciprocal(rmrec, rmsum)
    rm = pers.tile([BL, 3 * R], fp32, name="rm")
    for k in range(3):
        dve.tensor_mul(rm[:, k::3], rme[:, k::3], rmrec)

    rmT = []
    for k in range(3):
        t = colp.tile([R, BL], fp32, name=f"rmT{k}")
        tpose(t, rm[:, k::3], f"rmT{k}")
        rmT.append(t)

    # srk = rs / max(||rk||, eps)  -> [4, 8] columns
    rk_sq = ring.tile([BL, R * W], fp32, name="rk_sq", tag="big")
    act.activation(rk_sq, iv[:, _O_RK:_O_RK + R * W], Act.Square)
    rkn2 = ring.tile([BL, R], fp32, name="rkn2", tag="r4", bufs=4)
    dve.tensor_reduce(rkn2, rk_sq.rearrange("b (r w) -> b r w", w=W),
                      AX.X, Alu.add)
    rkn = ring.tile([BL, R], fp32, name="rkn", tag="r4b", bufs=4)
    act.activation(rkn, rkn2, Act.Sqrt)
    dve.tensor_scalar_max(rkn, rkn, 1e-8)
    rkninv = ring.tile([BL, R], fp32, name="rkninv", tag="r4", bufs=4)
    dve.reciprocal(rkninv, rkn)
    srk = ring.tile([BL, R], fp32, name="srk", tag="r4b", bufs=4)
    dve.tensor_mul(srk, rs_t, rkninv)
    srkT = colp.tile([R, BL], fp32, name="srkT")
    tpose(srkT, srk, "srkT")

    # rkT_r tiles [64, 8] for read-key matmuls
    rkT = []
    for r in range(R):
        t = colp.tile([W, BL], fp32, name=f"rkT{r}")
        tpose(t, iv[:, _O_RK + r * W:_O_RK + (r + 1) * W], f"rkT{r}")
        rkT.append(t)

    # wk norm inv [8,1]
    wk_sq = ring.tile([BL, W], fp32, name="wk_sq", tag="r64")
    wkn2 = ring.tile([BL, 1], fp32, name="wkn2", tag="b1", bufs=6)
    act.activation(wk_sq, iv[:, _O_WK:_O_WK + W], Act.Square, accum_out=wkn2)
    wkn = ring.tile([BL, 1], fp32, name="wkn", tag="b1", bufs=6)
    act.activation(wkn, wkn2, Act.Sqrt)
    dve.tensor_scalar_max(wkn, wkn, 1e-8)
    wkninv = pers.tile([BL, 1], fp32, name="wkninv")
    dve.reciprocal(wkninv, wkn)

    # ================= PHASE C: usage, write weighting =================
    usage = pers.tile([BL, N], fp32, name="usage")
    dma.dma_start(out=usage, in_=din["usage_vector"])
    for r in range(R):
        rw_r = big.tile([BL, N], fp32, name=f"rwr{r}", tag="bn", bufs=5)
        dma.dma_start(out=rw_r, in_=din["read_weights"][:, r, :])
        t_r = big.tile([BL, N], fp32, name=f"ret{r}", tag="bn", bufs=5)
        dve.tensor_scalar(t_r, rw_r, nfg[:, r:r + 1], 1.0, Alu.mult, Alu.add)
        dve.tensor_mul(usage, usage, t_r)
    dma.dma_start(out=dout["o_usage"], in_=usage)

    prec = pers.tile([BL, N], fp32, name="prec")
    dma.dma_start(out=prec, in_=din["precedence"])

    # memory matrix: M_sb[b] [128, (NB, W)] (persistent; updated in place)
    M_sb = []
    for b in range(BL):
        t = pers.tile([P, NB, W], fp32, name=f"M_{b}")
        dma.dma_start(out=t, in_=din["memory_matrix"][b].rearrange(
            "(k p) w -> p k w", p=P))
        M_sb.append(t)

    # write-content dot & norms (column layout, batch-major cols)
    scr64d = cpool.tile([P, W], fp32, name="scr64d")
    scr64a = cpool.tile([P, W], fp32, name="scr64a")
    scr64b = cpool.tile([P, W], fp32, name="scr64b")
    dotk = [colp.tile([P, BL], fp32, name=f"dotk{k}") for k in range(NB)]
    nm2k = [colp.tile([P, BL], fp32, name=f"nm2k{k}") for k in range(NB)]
    for b in range(BL):
        wk_rep = ring.tile([P, W], fp32, name=f"wkrep{b}", tag="wkrep")
        bcast(wk_rep, iv[b:b + 1, _O_WK:_O_WK + W], f"wk{b}")
        for k in range(NB):
            dve.scalar_tensor_tensor(scr64d, M_sb[b][:, k, :], 0.0, wk_rep,
                                     Alu.add, Alu.mult,
                                     accum_out=dotk[k][:, b:b + 1])
            act.activation(scr64a, M_sb[b][:, k, :], Act.Square,
                           accum_out=nm2k[k][:, b:b + 1])
    dot_b = big.tile([BL, N], fp32, name="dot_b", tag="bn", bufs=5)
    nm2_b = big.tile([BL, N], fp32, name="nm2_b", tag="bn", bufs=5)
    for k in range(NB):
        tpose(dot_b[:, k * P:(k + 1) * P], dotk[k], f"dk{k}")
        tpose(nm2_b[:, k * P:(k + 1) * P], nm2k[k], f"nk{k}")

    nminv = big.tile([BL, N], fp32, name="nminv", tag="bn", bufs=5)
    act.activation(nminv, nm2_b, Act.Sqrt)
    dve.tensor_scalar_max(nminv, nminv, 1e-8)
    dve.reciprocal(nminv, nminv)
    ws_eff = pers.tile([BL, 1], fp32, name="ws_eff")
    dve.tensor_mul(ws_eff, ws_t, wkninv)
    z_cw = big.tile([BL, N], fp32, name="z_cw", tag="bn", bufs=5)
    dve.scalar_tensor_tensor(z_cw, dot_b, ws_eff, nminv, Alu.mult, Alu.mult)
    zmax = ring.tile([BL, 1], fp32, name="zmax", tag="b1", bufs=6)
    dve.tensor_reduce(zmax, z_cw, AX.X, Alu.max)
    nzmax = pers.tile([BL, 1], fp32, name="nzmax")
    dve.tensor_scalar_mul(nzmax, zmax, -1.0)
    cw_e = big.tile([BL, N], fp32, name="cw_e", tag="bn", bufs=5)
    cw_s = ring.tile([BL, 1], fp32, name="cw_s", tag="b1", bufs=6)
    act.activation(cw_e, z_cw, Act.Exp, bias=nzmax, accum_out=cw_s)
    cw_r = pers.tile([BL, 1], fp32, name="cw_r")
    dve.reciprocal(cw_r, cw_s)
    cw = big.tile([BL, N], fp32, name="cw", tag="bn2")
    dve.tensor_scalar_mul(cw, cw_e, cw_r)

    # ---- allocation weighting (sort-free) ----
    lg = pers.tile([BL, N], fp32, name="lg")
    act.activation(lg, usage, Act.Ln)
    dve.tensor_scalar_max(lg, lg, NEG_CLAMP)
    uT = [colp.tile([P, BL], fp32, name=f"uT{k}") for k in range(NB)]
    for k in range(NB):
        tpose(uT[k], usage[:, k * P:(k + 1) * P], f"uT{k}")
    S_blk = [colp.tile([P, BL], fp32, name=f"S{k}") for k in range(NB)]
    scrN = cpool.tile([P, N], fp32, name="scrN")
    for b in range(BL):
        u_rep = big.tile([P, N], fp32, name=f"urep{b}", tag="urep", bufs=1)
        bcast(u_rep, usage[b:b + 1, :], f"u{b}")
        lg_rep = big.tile([P, N], fp32, name=f"lgrep{b}", tag="lgrep", bufs=1)
        bcast(lg_rep, lg[b:b + 1, :], f"lg{b}")
        for k in range(NB):
            dve.scalar_tensor_tensor(scrN, u_rep, uT[k][:, b:b + 1], lg_rep,
                                     Alu.is_lt, Alu.mult,
                                     accum_out=S_blk[k][:, b:b + 1])
    S_b = big.tile([BL, N], fp32, name="S_b", tag="bn", bufs=5)
    for k in range(NB):
        tpose(S_b[:, k * P:(k + 1) * P], S_blk[k], f"Sb{k}")
    alloc = big.tile([BL, N], fp32, name="alloc", tag="bn", bufs=5)
    act.activation(alloc, S_b, Act.Exp)
    dve.tensor_mul(alloc, alloc, usage)

    # ww = wg * (ag * alloc + (1 - ag) * cw)
    ww = pers.tile([BL, N], fp32, name="ww")
    t_a = big.tile([BL, N], fp32, name="t_a", tag="bn", bufs=5)
    dve.tensor_scalar_mul(t_a, alloc, ag_t)
    dve.scalar_tensor_tensor(ww, cw, nag_t, t_a, Alu.mult, Alu.add)
    dve.tensor_scalar_mul(ww, ww, wg_t)
    dma.dma_start(out=dout["o_ww"], in_=ww)

    # pnew = (1 - sum(ww)) * prec + ww
    wsum = ring.tile([BL, 1], fp32, name="wsum", tag="b1", bufs=6)
    dve.tensor_reduce(wsum, ww, AX.X, Alu.add)
    nws = ring.tile([BL, 1], fp32, name="nws", tag="b1", bufs=6)
    dve.tensor_scalar(nws, wsum, -1.0, 1.0, Alu.mult, Alu.add)
    pnew = big.tile([BL, N], fp32, name="pnew", tag="bn2")
    dve.scalar_tensor_tensor(pnew, prec, nws, ww, Alu.mult, Alu.add)
    dma.dma_start(out=dout["o_pnew"], in_=pnew)

    # transposed ww / (1-ww) / -ww columns
    wwT = [colp.tile([P, BL], fp32, name=f"wwT{k}") for k in range(NB)]
    sT = [colp.tile([P, BL], fp32, name=f"sT{k}") for k in range(NB)]
    nwT = [colp.tile([P, BL], fp32, name=f"nwT{k}") for k in range(NB)]
    for k in range(NB):
        tpose(wwT[k], ww[:, k * P:(k + 1) * P], f"wwT{k}")
        dve.tensor_scalar(sT[k], wwT[k], -1.0, 1.0, Alu.mult, Alu.add)
        dve.tensor_scalar_mul(nwT[k], wwT[k], -1.0)

    # write vector v = wv @ Wenc.T + benc
    wvT = ring.tile([W, BL], fp32, name="wvT", tag="wvT")
    tpose(wvT, iv[:, _O_WV:_O_WV + W], "wvT")
    v_ps = pp_small.tile([BL, W], fp32, name="v_ps", tag="tps")
    pe.matmul(v_ps, wvT, WencT, start=True, stop=False, skip_group_check=True)
    pe.matmul(v_ps, ones8, benc, start=False, stop=True, skip_group_check=True)
    v_sb = pers.tile([BL, W], fp32, name="v_sb")
    act.copy(v_sb, v_ps)

    # ================= PHASE D1: memory update + norms =================
    nm2r_k = [colp.tile([P, BL], fp32, name=f"nmr{k}") for k in range(NB)]
    for b in range(BL):
        ev_rep = ring.tile([P, W], fp32, name=f"evrep{b}", tag="evrep")
        bcast(ev_rep, ev[b:b + 1, :], f"ev{b}")
        v_rep = ring.tile([P, W], fp32, name=f"vrep{b}", tag="vrep")
        bcast(v_rep, v_sb[b:b + 1, :], f"v{b}")
        for k in range(NB):
            m_old = M_sb[b][:, k, :]
            t1 = ring.tile([P, W], fp32, name=f"mu1_{b}_{k}", tag="mu1")
            dve.scalar_tensor_tensor(t1, ev_rep, nwT[k][:, b:b + 1], m_old,
                                     Alu.mult, Alu.mult)
            t2 = ring.tile([P, W], fp32, name=f"mu2_{b}_{k}", tag="mu2")
            dve.scalar_tensor_tensor(t2, v_rep, wwT[k][:, b:b + 1], m_old,
                                     Alu.mult, Alu.add)
            dve.tensor_add(m_old, t1, t2)   # in-place: now mem_new
            act.activation(scr64b, m_old, Act.Square,
                           accum_out=nm2r_k[k][:, b:b + 1])
        dma.dma_start(out=dout["o_mem"][b].rearrange("(k p) w -> p k w", p=P),
                      in_=M_sb[b])

    # ================= PHASE D2: batched read norms =================
    nm2r_b = big.tile([BL, N], fp32, name="nm2r_b", tag="bn", bufs=5)
    for k in range(NB):
        tpose(nm2r_b[:, k * P:(k + 1) * P], nm2r_k[k], f"nmr{k}")
    nminv_r = pers.tile([BL, N], fp32, name="nminv_r")
    act.activation(nminv_r, nm2r_b, Act.Sqrt)
    dve.tensor_scalar_max(nminv_r, nminv_r, 1e-8)
    dve.reciprocal(nminv_r, nminv_r)

    rv_all = pers.tile([BL, R * W], fp32, name="rv_all")

    # ================= PHASE D3+E: per-batch read weights + link ========
    for b in range(BL):
        # rebuild memT from updated memory
        memT = big.tile([W, N], fp32, name=f"memT{b}", tag="memT")
        for k in range(NB):
            ppt = pp_small.tile([W, P], fp32, name=f"mT_{b}_{k}", tag="tps")
            pe.transpose(ppt, M_sb[b][:, k, :], ident)
            act.copy(memT[:, k * P:(k + 1) * P], ppt)
        # read keys
        rkT_b = ring.tile([W, R], fp32, name=f"rkTb{b}", tag="rkTb")
        for r in range(R):
            act.copy(rkT_b[:, r:r + 1], rkT[r][:, b:b + 1])
        dotR = pp_acc.tile([R, N], fp32, name=f"dotR{b}", tag="acc")
        for nch in range(2):
            pe.matmul(dotR[:, nch * 512:(nch + 1) * 512], rkT_b,
                      memT[:, nch * 512:(nch + 1) * 512],
                      start=True, stop=True, skip_group_check=True)
        nr_rep = big.tile([R, N], fp32, name=f"nrrep{b}", tag="nrrep")
        bcast(nr_rep, nminv_r[b:b + 1, :], f"nr{b}")
        zr = big.tile([R, N], fp32, name=f"zr{b}", tag="zr", bufs=4)
        dve.scalar_tensor_tensor(zr, dotR, srkT[:, b:b + 1], nr_rep,
                                 Alu.mult, Alu.mult)
        zrm = ring.tile([R, 1], fp32, name=f"zrm{b}", tag="zr1", bufs=6)
        dve.tensor_reduce(zrm, zr, AX.X, Alu.max)
        nzrm = ring.tile([R, 1], fp32, name=f"nzrm{b}", tag="zr1", bufs=6)
        dve.tensor_scalar_mul(nzrm, zrm, -1.0)
        ze = big.tile([R, N], fp32, name=f"ze{b}", tag="zr", bufs=4)
        zs = ring.tile([R, 1], fp32, name=f"zs{b}", tag="zr1", bufs=6)
        act.activation(ze, zr, Act.Exp, bias=nzrm, accum_out=zs)
        zrec = ring.tile([R, 1], fp32, name=f"zrec{b}", tag="zr1", bufs=6)
        dve.reciprocal(zrec, zs)
        rcw = big.tile([R, N], fp32, name=f"rcw{b}", tag="rcw")
        dve.tensor_scalar_mul(rcw, ze, zrec)

        # read weights transposed [128, (NB, R)]
        rw_b = big.tile([R, N], fp32, name=f"rwb{b}", tag="rwb")
        dma.dma_start(out=rw_b, in_=din["read_weights"][b])
        rwT = ring.tile([P, NB, R], fp32, name=f"rwT{b}", tag="rwT")
        for k in range(NB):
            ppt = pp_small.tile([P, R], fp32, name=f"rwt_{b}_{k}", tag="tps")
            pe.transpose(ppt, rw_b[:, k * P:(k + 1) * P], ident[:R, :R])
            act.copy(rwT[:, k, :], ppt)

        W_rep = big.tile([P, N], fp32, name=f"Wrep{b}", tag="Wrep")
        bcast(W_rep, ww[b:b + 1, :], f"W{b}")
        P_rep = big.tile([P, N], fp32, name=f"Prep{b}", tag="Prep")
        bcast(P_rep, prec[b:b + 1, :], f"P{b}")

        bwd_ps = pp_acc.tile([R, N], fp32, name=f"bwdps{b}", tag="acc")
        fwd_ps = pp_acc.tile([R, N], fp32, name=f"fwdps{b}", tag="acc")

        for k in range(NB):
            lt = big.tile([P, N], fp32, name=f"lt_{b}_{k}", tag="lt", bufs=3)
            dma.dma_start(out=lt, in_=din["link"][b, k * P:(k + 1) * P, :])
            # lt = (ww_row - (1-ww_i)) * link   (in-place)
            dve.scalar_tensor_tensor(lt, W_rep, sT[k][:, b:b + 1], lt,
                                     Alu.subtract, Alu.mult)
            # lt = (prec_row * ww_i) - lt  -> Lnew (in-place)
            dve.scalar_tensor_tensor(lt, P_rep, wwT[k][:, b:b + 1], lt,
                                     Alu.mult, Alu.subtract)
            # zero diagonal block stripe
            gps.affine_select(out=lt[:, k * P:(k + 1) * P],
                              in_=lt[:, k * P:(k + 1) * P],
                              compare_op=Alu.not_equal, fill=0.0,
                              base=0, pattern=[[-1, P]], channel_multiplier=1)
            dma.dma_start(out=dout["o_Lnew"][b, k * P:(k + 1) * P, :], in_=lt)
            # bwd += rwT_k.T @ lt
            for nch in range(2):
                pe.matmul(bwd_ps[:, nch * 512:(nch + 1) * 512], rwT[:, k, :],
                          lt[:, nch * 512:(nch + 1) * 512],
                          start=(k == 0), stop=(k == NB - 1),
                          skip_group_check=True)
            # transpose lt -> ltT, then fwd_k = sum_c rwT_c.T @ ltT_c
            pbig = pp_acc.tile([P, N], fp32, name=f"pbig_{b}_{k}", tag="pbig",
                               bufs=1)
            for c in range(NB):
                pe.transpose(pbig[:, c * P:(c + 1) * P],
                             lt[:, c * P:(c + 1) * P], ident)
            ltT = big.tile([P, N], fp32, name=f"ltT_{b}_{k}", tag="ltT")
            act.copy(ltT, pbig)
            for c in range(NB):
                pe.matmul(fwd_ps[:, k * P:(k + 1) * P], rwT[:, c, :],
                          ltT[:, c * P:(c + 1) * P],
                          start=(c == 0), stop=(c == NB - 1),
                          skip_group_check=True)

        # wr = rm0*bwd + rm1*rcw + rm2*fwd
        t1 = big.tile([R, N], fp32, name=f"wr1_{b}", tag="zr", bufs=4)
        dve.tensor_scalar_mul(t1, bwd_ps, rmT[0][:, b:b + 1])
        t2 = big.tile([R, N], fp32, name=f"wr2_{b}", tag="zr", bufs=4)
        dve.scalar_tensor_tensor(t2, rcw, rmT[1][:, b:b + 1], t1,
                                 Alu.mult, Alu.add)
        wr = big.tile([R, N], fp32, name=f"wr{b}", tag="wr")
        dve.scalar_tensor_tensor(wr, fwd_ps, rmT[2][:, b:b + 1], t2,
                                 Alu.mult, Alu.add)
        dma.dma_start(out=dout["o_wr"][b], in_=wr)

        # rv = wr @ mem  ([4, 64])
        rv_ps = pp_acc.tile([R, W], fp32, name=f"rvps{b}", tag="acc")
        wrT = ring.tile([P, NB, R], fp32, name=f"wrT{b}", tag="wrT")
        for k in range(NB):
            ppt = pp_small.tile([P, R], fp32, name=f"wrt_{b}_{k}", tag="tps")
            pe.transpose(ppt, wr[:, k * P:(k + 1) * P], ident[:R, :R])
            act.copy(wrT[:, k, :], ppt)
            pe.matmul(rv_ps, wrT[:, k, :], M_sb[b][:, k, :],
                      start=(k == 0), stop=(k == NB - 1), skip_group_check=True)
        rv_sb = ring.tile([R, W], fp32, name=f"rv{b}", tag="rv")
        act.copy(rv_sb, rv_ps)
        dma.dma_start(out=dout["o_rv"][b], in_=rv_sb)
        dma.dma_start(out=rv_all[b:b + 1, :], in_=rv_sb)

    # ================= PHASE F: output projection =================
    out_ps = pp_acc.tile([BL, W], fp32, name="out_ps", tag="acc")
    cat_chunks = []
    for c in range(2):
        t = ring.tile([P, BL], fp32, name=f"hT{c}", tag="catT", bufs=4)
        tpose(t, h3[:, c * P:(c + 1) * P], f"hT{c}")
        cat_chunks.append(t)
    for c in range(2):
        t = ring.tile([P, BL], fp32, name=f"rvT{c}", tag="catT", bufs=4)
        tpose(t, rv_all[:, c * P:(c + 1) * P], f"rvT{c}")
        cat_chunks.append(t)
    for c in range(4):
        pe.matmul(out_ps, cat_chunks[c], WoutT[c], start=(c == 0),
                  stop=False, skip_group_check=True)
    pe.matmul(out_ps, ones8, bout, start=False, stop=True,
              skip_group_check=True)
    out_sb = ring.tile([BL, W], fp32, name="out_sb", tag="r64")
    act.copy(out_sb, out_ps)
    dma.dma_start(out=dout["o_out"], in_=out_sb)

    for p_ in [big, pp_acc, pp_small, ring, colp, pers, cpool]:
        p_.release()


def _get_nc():
    if "nc" not in _CACHE:
        _CACHE["nc"] = _build()
    return _CACHE["nc"]


def _run(inputs, trace=False):
    from concourse.bass_utils import run_bass_kernel_spmd
    nc = _get_nc()
    rep_names = ["Wih0", "bih0", "bhh0", "Wih1", "bih1", "bhh1",
                 "Wih2", "bih2", "bhh2", "Wint", "bint", "Wenc", "benc",
                 "Wout", "bout"]
    shard_names = ["input_vector", "memory_matrix", "read_weights",
                   "usage_vector", "link", "precedence", "read_vectors"]
    in_maps = []
    for c in range(NCORES):
        m = {}
        for k in shard_names:
            m[k] = np.ascontiguousarray(
                np.asarray(inputs[k], np.float32)[c * BL:(c + 1) * BL])
        for k in rep_names:
            m[k] = np.ascontiguousarray(np.asarray(inputs[k], np.float32))
        in_maps.append(m)
    res = run_bass_kernel_spmd(nc, in_maps, list(range(NCORES)), trace=trace)
    outs = {}
    for name in ["o_out", "o_mem", "o_wr", "o_ww", "o_usage", "o_Lnew",
                 "o_pnew", "o_rv"]:
        outs[name] = np.concatenate([res.results[c][name]
                                     for c in range(NCORES)], axis=0)
    return outs, res


def kernel(**inputs):
    outs, _ = _run(inputs, trace=False)
    return (outs["o_out"], outs["o_mem"], outs["o_wr"], outs["o_ww"],
            outs["o_usage"], outs["o_Lnew"], outs["o_pnew"], outs["o_rv"])
